# revision 12
# baseline (speedup 1.0000x reference)
"""Trainium2 Bass kernel for nn_BCNet: three-way low-rank bilinear net.

reference:
  v_ = relu(v @ Wv.T + bv)            # (B, NV, HK)
  q_ = relu(q @ Wq.T + bq)            # (B, NQ, HK)
  logits = einsum('hk,bvk,bqk->bhvq', h_mat, v_, q_) + h_bias

Sharding: data-parallel over batch, 4 batch items per core (8 cores).
All matmuls in bf16 with fp32 PSUM accumulation.

The kernel is PE-streaming-bound (639,700 moving columns = 266.5us at
2.4 GHz); everything else is engineered to hide behind that stream:
 - few, large DMAs (issue cost ~0.7us each; teardown scales with count)
 - critical first chunks issued first, spread over 4 issue engines
 - PE warm-up matmuls on a memset scratch tile (HAM un-throttle early)
 - qh built with broadcast tensor_tensor (48 DVE ops, not 384)
 - stage C reuses each vact weight tile for both nh halves
 - output stored as bf16, one DMA per (b, vc); host upcasts

Host prep per core:
  vT   (4, 2048, 512) bf16  : v[b].T per batch item
  qT   (1024, 512)    bf16  : q[4c:4c+4] transposed+stacked, cols = b*128+q
  WvT  (2048, 1536)   bf16
  WqT  (1024, 1536)   bf16
  bias (128, 12+12+1024) f32 : [bvT | bqT | hb broadcast]
  hm   (128, 12, 8) bf16 : h_mat[h, jc*128+p]
Device output per core: out (4, 512, 1024) bf16, cols = h*128+q.
Host post: concat -> (32, 512, 8, 128) -> transpose -> (32, 8, 512, 128).
"""

import numpy as np

B, NV, NQ = 32, 512, 128
V_DIM, Q_DIM, HK, H_OUT = 2048, 1024, 1536, 8
N_CORES = 8
BPC = B // N_CORES          # 4 batch items per core
JC = HK // 128              # 12 k-chunks
DCV = V_DIM // 128          # 16 contraction chunks for v
DCQ = Q_DIM // 128          # 8 contraction chunks for q
VC = NV // 128              # 4 v-chunks

_CACHE = {}


def _build_nc():
    import concourse.tile as tile
    from concourse import bacc, mybir
    from contextlib import ExitStack

    bf16 = mybir.dt.bfloat16
    f32 = mybir.dt.float32

    nc = bacc.Bacc()

    vT = nc.declare_dram_parameter("vT", [BPC, V_DIM, NV], bf16, isOutput=False)
    qT = nc.declare_dram_parameter("qT", [Q_DIM, BPC * NQ], bf16, isOutput=False)
    WvT = nc.declare_dram_parameter("WvT", [V_DIM, HK], bf16, isOutput=False)
    WqT = nc.declare_dram_parameter("WqT", [Q_DIM, HK], bf16, isOutput=False)
    bias = nc.declare_dram_parameter("bias", [128, 2 * JC + H_OUT * NQ], f32, isOutput=False)
    hm = nc.declare_dram_parameter("hm", [128, JC, H_OUT], bf16, isOutput=False)
    out = nc.declare_dram_parameter("out", [BPC, NV, H_OUT * NQ], bf16, isOutput=True)

    with ExitStack() as ctx:
        tc = ctx.enter_context(tile.TileContext(nc))
        consts = ctx.enter_context(tc.tile_pool(name="consts", bufs=1))
        qpool = ctx.enter_context(tc.tile_pool(name="qpool", bufs=1))
        vin = ctx.enter_context(tc.tile_pool(name="vin", bufs=2))
        vact = ctx.enter_context(tc.tile_pool(name="vact", bufs=2))
        qhp = ctx.enter_context(tc.tile_pool(name="qhp", bufs=1))
        outp = ctx.enter_context(tc.tile_pool(name="outp", bufs=3))
        ps = ctx.enter_context(tc.tile_pool(name="ps", bufs=8, space="PSUM"))

        # ---- PE warm-up: a few matmuls on zeroed scratch so the HAM
        # un-throttles (K=8/8) before the first real matmul arrives.
        scratch = consts.tile([128, 8 + 512], bf16)
        nc.gpsimd.memset(scratch, 0.0)
        ps_warm = ps.tile([8, 512], f32, tag="ps", name="warm")
        for _ in range(6):
            nc.tensor.matmul(ps_warm, lhsT=scratch[:, 0:8], rhs=scratch[:, 8:520],
                             start=True, stop=True)

        # ---- input DMAs. Three issue rings (sync/scalar HWDGE, gpsimd
        # SWDGE) share ~358 GB/s of HBM bandwidth; SDMA engines
        # round-robin rings at packet granularity, so bulk transfers
        # issued early starve the critical stream. Order of need:
        # Wq+qT (stage B, ~10us) -> Wv block0 + vT[0] (stage A start,
        # ~30us) -> Wv blocks 1-2 -> vT[1..3]/h_bias. Later transfers
        # are gated behind compute milestones via tiny copies.
        qT_r = qT.rearrange("(d p) n -> p d n", p=128)
        qt_sb = qpool.tile([128, DCQ, BPC * NQ], bf16)
        WqT_r = WqT.rearrange("(d p) j -> p d j", p=128)
        wq_sb = consts.tile([128, DCQ, HK], bf16)
        # Sync ring: Wq by j-block (B group g consumes wq[:, :, g*512:]).
        nc.sync.dma_start(out=wq_sb[:, 0:4, 0:512], in_=WqT_r[:, 0:4, 0:512])
        nc.sync.dma_start(out=wq_sb[:, 4:DCQ, 0:512], in_=WqT_r[:, 4:DCQ, 0:512])
        nc.sync.dma_start(out=wq_sb[:, :, 512:1024], in_=WqT_r[:, :, 512:1024])
        nc.sync.dma_start(out=wq_sb[:, :, 1024:1536], in_=WqT_r[:, :, 1024:1536])
        # GpSimd ring: qT (B's moving operand) + small constants.
        nc.gpsimd.dma_start(out=qt_sb[:, 0:4, :], in_=qT_r[:, 0:4, :])
        nc.gpsimd.dma_start(out=qt_sb[:, 4:DCQ, :], in_=qT_r[:, 4:DCQ, :])
        bias_sb = consts.tile([128, 2 * JC + H_OUT * NQ], f32)
        hm_sb = consts.tile([128, JC, H_OUT], bf16)
        nc.gpsimd.dma_start(out=bias_sb[:, 0:2 * JC], in_=bias[:, 0:2 * JC])
        nc.gpsimd.dma_start(out=hm_sb, in_=hm[:, :, :])
        bv_sb = bias_sb[:, 0:JC]
        bq_sb = bias_sb[:, JC:2 * JC]
        hb_sb = bias_sb[:, 2 * JC:]
        # Scalar ring: Wv by j-block halves (A group g needs cols
        # g*512:(g+1)*512 of every d), gated behind qT's first chunk.
        WvT_r = WvT.rearrange("(d p) j -> p d j", p=128)
        wv_sb = consts.tile([128, DCV, HK], bf16)
        sscr = consts.tile([128, 8], bf16)
        nc.scalar.copy(sscr[:, 0:1], qt_sb[:, 0, 0:1])
        for jb in range(3):
            c0, c1 = jb * 512, (jb + 1) * 512
            nc.scalar.dma_start(out=wv_sb[:, 0:8, c0:c1], in_=WvT_r[:, 0:8, c0:c1])
            nc.scalar.dma_start(out=wv_sb[:, 8:DCV, c0:c1], in_=WvT_r[:, 8:DCV, c0:c1])

        # ---- stage B: q_ = relu(q @ Wq.T + bq), all 4 b at once ----
        # j-quads rotating over the shared 8-bank psum pool.
        qact_sb = qpool.tile([128, JC, BPC * NQ], bf16)
        for jg in range(0, JC, 4):
            pss = [ps.tile([128, BPC * NQ], f32, tag="ps", name=f"psB{jg}_{i}") for i in range(4)]
            for d in range(DCQ):
                for ji in range(4):
                    j = jg + ji
                    nc.tensor.matmul(
                        pss[ji],
                        lhsT=wq_sb[:, d, j * 128:(j + 1) * 128],
                        rhs=qt_sb[:, d, :],
                        start=(d == 0),
                        stop=(d == DCQ - 1),
                    )
            for ji in range(4):
                j = jg + ji
                nc.scalar.activation(
                    out=qact_sb[:, j, :],
                    in_=pss[ji],
                    func=mybir.ActivationFunctionType.Relu,
                    bias=bq_sb[:, j:j + 1],
                    scale=1.0,
                )

        # vT[0] on the gpsimd ring, gated behind B's first activation so
        # Wq/qT/Wv-block0 own the early bandwidth.
        gscr = consts.tile([128, 8], bf16)
        vt0_sb = vin.tile([128, DCV, NV], bf16, tag="vt", name="vt0")
        vT0_r = vT[0].rearrange("(d p) n -> p d n", p=128)
        nc.gpsimd.tensor_copy(gscr[:, 0:1], qact_sb[:, 0, 0:1])
        nc.gpsimd.dma_start(out=vt0_sb[:, 0:8, :], in_=vT0_r[:, 0:8, :])
        nc.gpsimd.dma_start(out=vt0_sb[:, 8:DCV, :], in_=vT0_r[:, 8:DCV, :])

        vt_tiles = {0: vt0_sb}
        for b in range(BPC):
            # ---- Qh[b][k, (h,q')] = q_[k, b*128+q'] * h_mat[h, k] ----
            # one broadcast tensor_tensor per j (DVE), not one per (j,h)
            qh_sb = qhp.tile([128, JC, H_OUT, NQ], bf16, tag="qh")
            for j in range(JC):
                nc.vector.tensor_mul(
                    qh_sb[:, j],
                    qact_sb[:, j, b * NQ:(b + 1) * NQ].unsqueeze(1).broadcast_to((128, H_OUT, NQ)),
                    hm_sb[:, j, :].unsqueeze(2).broadcast_to((128, H_OUT, NQ)),
                )

            # ---- stage A: v_[b] = relu(v[b] @ Wv.T + bv), transposed layout
            vt_sb = vt_tiles[b]
            vact_sb = vact.tile([128, JC, NV], bf16, tag="vact")
            for jg in range(0, JC, 4):
                pss = [ps.tile([128, NV], f32, tag="ps", name=f"psA{b}_{jg}_{i}") for i in range(4)]
                for d in range(DCV):
                    for ji in range(4):
                        j = jg + ji
                        nc.tensor.matmul(
                            pss[ji],
                            lhsT=wv_sb[:, d, j * 128:(j + 1) * 128],
                            rhs=vt_sb[:, d, :],
                            start=(d == 0),
                            stop=(d == DCV - 1),
                        )
                for ji in range(4):
                    j = jg + ji
                    nc.scalar.activation(
                        out=vact_sb[:, j, :],
                        in_=pss[ji],
                        func=mybir.ActivationFunctionType.Relu,
                        bias=bv_sb[:, j:j + 1],
                        scale=1.0,
                    )

            # prefetch next batch's vT on gpsimd, gated behind this
            # batch's first A-activation (ahead of C's gpsimd stores).
            if b + 1 < BPC:
                nvt = vin.tile([128, DCV, NV], bf16, tag="vt")
                vT_r = vT[b + 1].rearrange("(d p) n -> p d n", p=128)
                nc.gpsimd.tensor_copy(gscr[:, b + 1:b + 2], vact_sb[:, 0, 0:1])
                nc.gpsimd.dma_start(out=nvt, in_=vT_r)
                vt_tiles[b + 1] = nvt
            if b == 0:
                # h_bias after vT[1]: needed by C(b0)'s adds (~80us).
                nc.gpsimd.dma_start(out=bias_sb[:, 2 * JC:], in_=bias[:, 2 * JC:])

            # ---- stage C: logits[b] = v_[b] @ Qh[b] (contract over k)
            # j outer / nh inner: each vact weight tile loads once for
            # both nh halves; two 512-wide psum tiles per vc.
            engs = [nc.sync, nc.gpsimd, nc.scalar]
            for vc in range(VC):
                pos = [ps.tile([128, H_OUT * NQ // 2], f32, tag="ps",
                               name=f"psC{b}_{vc}_{nh}") for nh in range(2)]
                for j in range(JC):
                    for nh in range(2):
                        nc.tensor.matmul(
                            pos[nh],
                            lhsT=vact_sb[:, j, vc * 128:(vc + 1) * 128],
                            rhs=qh_sb[:, j, nh * 4:(nh + 1) * 4, :],
                            start=(j == 0),
                            stop=(j == JC - 1),
                        )
                o_sb = outp.tile([128, H_OUT * NQ], bf16, tag="osb")
                for nh in range(2):
                    sl = slice(nh * 512, (nh + 1) * 512)
                    nc.vector.tensor_add(o_sb[:, sl], pos[nh], hb_sb[:, sl])
                engs[(b * VC + vc) % 3].dma_start(
                    out=out[b, vc * 128:(vc + 1) * 128, :], in_=o_sb
                )

    nc.compile()
    return nc


def kernel(v, q, Wv, bv, Wq, bq, h_mat, h_bias):
    import ml_dtypes
    from concourse import bass_utils

    bf16 = ml_dtypes.bfloat16

    if "nc" not in _CACHE:
        _CACHE["nc"] = _build_nc()
    nc = _CACHE["nc"]

    v = np.asarray(v, dtype=np.float32)
    q = np.asarray(q, dtype=np.float32)
    Wv = np.asarray(Wv, dtype=np.float32)
    Wq = np.asarray(Wq, dtype=np.float32)
    bv = np.asarray(bv, dtype=np.float32)
    bq = np.asarray(bq, dtype=np.float32)
    h_mat = np.asarray(h_mat, dtype=np.float32)
    h_bias = np.asarray(h_bias, dtype=np.float32)

    vT = np.ascontiguousarray(v.transpose(0, 2, 1)).astype(bf16)      # (B, 2048, 512)
    WvT = np.ascontiguousarray(Wv.T).astype(bf16)                     # (2048, 1536)
    WqT = np.ascontiguousarray(Wq.T).astype(bf16)                     # (1024, 1536)
    bvT = np.ascontiguousarray(bv.reshape(JC, 128).T)                 # (128, 12)
    bqT = np.ascontiguousarray(bq.reshape(JC, 128).T)
    # hm[p, jc, h] = h_mat[h, jc*128+p]
    hmP = np.ascontiguousarray(
        h_mat.reshape(H_OUT, JC, 128).transpose(2, 1, 0)).astype(bf16)
    hbB = np.broadcast_to(np.repeat(h_bias, NQ)[None, :], (128, H_OUT * NQ))
    biasP = np.ascontiguousarray(
        np.concatenate([bvT, bqT, hbB], axis=1)).astype(np.float32)

    in_maps = []
    for c in range(N_CORES):
        bs = slice(BPC * c, BPC * (c + 1))
        qTc = np.ascontiguousarray(
            q[bs].transpose(2, 0, 1).reshape(Q_DIM, BPC * NQ)
        ).astype(bf16)
        in_maps.append({
            "vT": vT[bs],
            "qT": qTc,
            "WvT": WvT,
            "WqT": WqT,
            "bias": biasP,
            "hm": hmP,
        })

    res = bass_utils.run_bass_kernel_spmd(nc, in_maps, list(range(N_CORES)))
    outs = np.concatenate(
        [np.asarray(res.results[c]["out"]).astype(np.float32) for c in range(N_CORES)],
        axis=0,
    )
    # (32, 512, 1024) -> (32, 512, 8, 128) -> (32, 8, 512, 128)
    logits = outs.reshape(B, NV, H_OUT, NQ).transpose(0, 2, 1, 3)
    return np.ascontiguousarray(logits)


# revision 14
# speedup vs baseline: 1.0025x; 1.0025x over previous
"""Trainium2 Bass kernel for nn_BCNet: three-way low-rank bilinear net.

reference:
  v_ = relu(v @ Wv.T + bv)            # (B, NV, HK)
  q_ = relu(q @ Wq.T + bq)            # (B, NQ, HK)
  logits = einsum('hk,bvk,bqk->bhvq', h_mat, v_, q_) + h_bias

Sharding: data-parallel over batch, 4 batch items per core (8 cores).
All matmuls in bf16 with fp32 PSUM accumulation.

The kernel is PE-streaming-bound (639,700 moving columns = 266.5us at
2.4 GHz); everything else is engineered to hide behind that stream:
 - few, large DMAs (issue cost ~0.7us each; teardown scales with count)
 - critical first chunks issued first, spread over 4 issue engines
 - PE warm-up matmuls on a memset scratch tile (HAM un-throttle early)
 - qh built with broadcast tensor_tensor (48 DVE ops, not 384)
 - stage C reuses each vact weight tile for both nh halves
 - output stored as bf16, one DMA per (b, vc); host upcasts

Host prep per core:
  vT   (4, 2048, 512) bf16  : v[b].T per batch item
  qT   (1024, 512)    bf16  : q[4c:4c+4] transposed+stacked, cols = b*128+q
  WvT  (2048, 1536)   bf16
  WqT  (1024, 1536)   bf16
  bias (128, 12+12+1024) f32 : [bvT | bqT | hb broadcast]
  hm   (128, 12, 8) bf16 : h_mat[h, jc*128+p]
Device output per core: out (4, 512, 1024) bf16, cols = h*128+q.
Host post: concat -> (32, 512, 8, 128) -> transpose -> (32, 8, 512, 128).
"""

import numpy as np

B, NV, NQ = 32, 512, 128
V_DIM, Q_DIM, HK, H_OUT = 2048, 1024, 1536, 8
N_CORES = 8
BPC = B // N_CORES          # 4 batch items per core
JC = HK // 128              # 12 k-chunks
DCV = V_DIM // 128          # 16 contraction chunks for v
DCQ = Q_DIM // 128          # 8 contraction chunks for q
VC = NV // 128              # 4 v-chunks

_CACHE = {}


def _build_nc():
    import concourse.tile as tile
    from concourse import bacc, mybir
    from contextlib import ExitStack

    bf16 = mybir.dt.bfloat16
    f32 = mybir.dt.float32

    nc = bacc.Bacc()

    vT = nc.declare_dram_parameter("vT", [BPC, V_DIM, NV], bf16, isOutput=False)
    qT = nc.declare_dram_parameter("qT", [Q_DIM, BPC * NQ], bf16, isOutput=False)
    WvT = nc.declare_dram_parameter("WvT", [V_DIM, HK], bf16, isOutput=False)
    WqT = nc.declare_dram_parameter("WqT", [Q_DIM, HK], bf16, isOutput=False)
    bias = nc.declare_dram_parameter("bias", [128, 2 * JC + H_OUT * NQ], f32, isOutput=False)
    hm = nc.declare_dram_parameter("hm", [128, JC, H_OUT], bf16, isOutput=False)
    out = nc.declare_dram_parameter("out", [BPC, NV, H_OUT * NQ], bf16, isOutput=True)

    with ExitStack() as ctx:
        tc = ctx.enter_context(tile.TileContext(nc))
        consts = ctx.enter_context(tc.tile_pool(name="consts", bufs=1))
        qpool = ctx.enter_context(tc.tile_pool(name="qpool", bufs=1))
        vin = ctx.enter_context(tc.tile_pool(name="vin", bufs=2))
        vact = ctx.enter_context(tc.tile_pool(name="vact", bufs=2))
        qhp = ctx.enter_context(tc.tile_pool(name="qhp", bufs=1))
        outp = ctx.enter_context(tc.tile_pool(name="outp", bufs=3))
        ps = ctx.enter_context(tc.tile_pool(name="ps", bufs=8, space="PSUM"))

        # ---- PE warm-up: a few matmuls on zeroed scratch so the HAM
        # un-throttles (K=8/8) before the first real matmul arrives.
        scratch = consts.tile([128, 8 + 512], bf16)
        nc.gpsimd.memset(scratch, 0.0)
        ps_warm = ps.tile([8, 512], f32, tag="ps", name="warm")
        for _ in range(6):
            nc.tensor.matmul(ps_warm, lhsT=scratch[:, 0:8], rhs=scratch[:, 8:520],
                             start=True, stop=True)

        # ---- input DMAs. Three issue rings (sync/scalar HWDGE, gpsimd
        # SWDGE) share ~358 GB/s of HBM bandwidth; SDMA engines
        # round-robin rings at packet granularity, so bulk transfers
        # issued early starve the critical stream. Order of need:
        # Wq+qT (stage B, ~10us) -> Wv block0 + vT[0] (stage A start,
        # ~30us) -> Wv blocks 1-2 -> vT[1..3]/h_bias. Later transfers
        # are gated behind compute milestones via tiny copies.
        qT_r = qT.rearrange("(d p) n -> p d n", p=128)
        qt_sb = qpool.tile([128, DCQ, BPC * NQ], bf16)
        WqT_r = WqT.rearrange("(d p) j -> p d j", p=128)
        wq_sb = consts.tile([128, DCQ, HK], bf16)
        # Sync ring, in stage-B consumption order: qT and Wq j-block 0
        # interleaved (B's first MMs), then Wq j-blocks 1-2.
        nc.sync.dma_start(out=qt_sb[:, 0:4, :], in_=qT_r[:, 0:4, :])
        nc.sync.dma_start(out=wq_sb[:, 0:4, 0:512], in_=WqT_r[:, 0:4, 0:512])
        nc.sync.dma_start(out=qt_sb[:, 4:DCQ, :], in_=qT_r[:, 4:DCQ, :])
        nc.sync.dma_start(out=wq_sb[:, 4:DCQ, 0:512], in_=WqT_r[:, 4:DCQ, 0:512])
        nc.sync.dma_start(out=wq_sb[:, :, 512:1024], in_=WqT_r[:, :, 512:1024])
        nc.sync.dma_start(out=wq_sb[:, :, 1024:1536], in_=WqT_r[:, :, 1024:1536])
        # GpSimd ring: small constants only (vT gated later).
        bias_sb = consts.tile([128, 2 * JC + H_OUT * NQ], f32)
        hm_sb = consts.tile([128, JC, H_OUT], bf16)
        nc.gpsimd.dma_start(out=bias_sb[:, 0:2 * JC], in_=bias[:, 0:2 * JC])
        nc.gpsimd.dma_start(out=hm_sb, in_=hm[:, :, :])
        bv_sb = bias_sb[:, 0:JC]
        bq_sb = bias_sb[:, JC:2 * JC]
        hb_sb = bias_sb[:, 2 * JC:]
        # Scalar ring: Wv, gated behind qT so Wq/qT own the early
        # bandwidth. Block 0 in d-quarters (stage A consumes the first
        # group's d-chunks progressively); blocks 1-2 whole.
        WvT_r = WvT.rearrange("(d p) j -> p d j", p=128)
        wv_sb = consts.tile([128, DCV, HK], bf16)
        sscr = consts.tile([128, 8], bf16)
        nc.scalar.copy(sscr[:, 0:1], qt_sb[:, 4, 0:1])
        for dq in range(4):
            nc.scalar.dma_start(out=wv_sb[:, 4 * dq:4 * dq + 4, 0:512],
                                in_=WvT_r[:, 4 * dq:4 * dq + 4, 0:512])
        nc.scalar.dma_start(out=wv_sb[:, :, 512:1024], in_=WvT_r[:, :, 512:1024])
        nc.scalar.dma_start(out=wv_sb[:, :, 1024:1536], in_=WvT_r[:, :, 1024:1536])

        # ---- stage B: q_ = relu(q @ Wq.T + bq), all 4 b at once ----
        # j-quads rotating over the shared 8-bank psum pool.
        qact_sb = qpool.tile([128, JC, BPC * NQ], bf16)
        for jg in range(0, JC, 4):
            pss = [ps.tile([128, BPC * NQ], f32, tag="ps", name=f"psB{jg}_{i}") for i in range(4)]
            for d in range(DCQ):
                for ji in range(4):
                    j = jg + ji
                    nc.tensor.matmul(
                        pss[ji],
                        lhsT=wq_sb[:, d, j * 128:(j + 1) * 128],
                        rhs=qt_sb[:, d, :],
                        start=(d == 0),
                        stop=(d == DCQ - 1),
                    )
            for ji in range(4):
                j = jg + ji
                nc.scalar.activation(
                    out=qact_sb[:, j, :],
                    in_=pss[ji],
                    func=mybir.ActivationFunctionType.Relu,
                    bias=bq_sb[:, j:j + 1],
                    scale=1.0,
                )

        # vT[0] on the gpsimd ring, gated behind B's first activation so
        # Wq/qT/Wv-block0 own the early bandwidth.
        gscr = consts.tile([128, 8], bf16)
        vt0_sb = vin.tile([128, DCV, NV], bf16, tag="vt", name="vt0")
        vT0_r = vT[0].rearrange("(d p) n -> p d n", p=128)
        nc.gpsimd.tensor_copy(gscr[:, 0:1], qact_sb[:, 0, 0:1])
        for dq in range(4):
            nc.gpsimd.dma_start(out=vt0_sb[:, 4 * dq:4 * dq + 4, :],
                                in_=vT0_r[:, 4 * dq:4 * dq + 4, :])

        vt_tiles = {0: vt0_sb}
        for b in range(BPC):
            # ---- Qh[b][k, (h,q')] = q_[k, b*128+q'] * h_mat[h, k] ----
            # one broadcast tensor_tensor per j (DVE), not one per (j,h)
            qh_sb = qhp.tile([128, JC, H_OUT, NQ], bf16, tag="qh")
            for j in range(JC):
                nc.vector.tensor_mul(
                    qh_sb[:, j],
                    qact_sb[:, j, b * NQ:(b + 1) * NQ].unsqueeze(1).broadcast_to((128, H_OUT, NQ)),
                    hm_sb[:, j, :].unsqueeze(2).broadcast_to((128, H_OUT, NQ)),
                )

            # ---- stage A: v_[b] = relu(v[b] @ Wv.T + bv), transposed layout
            vt_sb = vt_tiles[b]
            vact_sb = vact.tile([128, JC, NV], bf16, tag="vact")
            for jg in range(0, JC, 4):
                pss = [ps.tile([128, NV], f32, tag="ps", name=f"psA{b}_{jg}_{i}") for i in range(4)]
                for d in range(DCV):
                    for ji in range(4):
                        j = jg + ji
                        nc.tensor.matmul(
                            pss[ji],
                            lhsT=wv_sb[:, d, j * 128:(j + 1) * 128],
                            rhs=vt_sb[:, d, :],
                            start=(d == 0),
                            stop=(d == DCV - 1),
                        )
                for ji in range(4):
                    j = jg + ji
                    nc.scalar.activation(
                        out=vact_sb[:, j, :],
                        in_=pss[ji],
                        func=mybir.ActivationFunctionType.Relu,
                        bias=bv_sb[:, j:j + 1],
                        scale=1.0,
                    )

            # prefetch next batch's vT on gpsimd, gated behind this
            # batch's first A-activation (ahead of C's gpsimd stores).
            if b + 1 < BPC:
                nvt = vin.tile([128, DCV, NV], bf16, tag="vt")
                vT_r = vT[b + 1].rearrange("(d p) n -> p d n", p=128)
                nc.gpsimd.tensor_copy(gscr[:, b + 1:b + 2], vact_sb[:, 0, 0:1])
                nc.gpsimd.dma_start(out=nvt, in_=vT_r)
                vt_tiles[b + 1] = nvt
            if b == 0:
                # h_bias after vT[1]: needed by C(b0)'s adds (~80us).
                nc.gpsimd.dma_start(out=bias_sb[:, 2 * JC:], in_=bias[:, 2 * JC:])

            # ---- stage C: logits[b] = v_[b] @ Qh[b] (contract over k)
            # j outer / nh inner: each vact weight tile loads once for
            # both nh halves; two 512-wide psum tiles per vc.
            engs = [nc.sync, nc.gpsimd, nc.scalar]
            for vc in range(VC):
                pos = [ps.tile([128, H_OUT * NQ // 2], f32, tag="ps",
                               name=f"psC{b}_{vc}_{nh}") for nh in range(2)]
                for j in range(JC):
                    for nh in range(2):
                        nc.tensor.matmul(
                            pos[nh],
                            lhsT=vact_sb[:, j, vc * 128:(vc + 1) * 128],
                            rhs=qh_sb[:, j, nh * 4:(nh + 1) * 4, :],
                            start=(j == 0),
                            stop=(j == JC - 1),
                        )
                o_sb = outp.tile([128, H_OUT * NQ], bf16, tag="osb")
                for nh in range(2):
                    sl = slice(nh * 512, (nh + 1) * 512)
                    nc.vector.tensor_add(o_sb[:, sl], pos[nh], hb_sb[:, sl])
                engs[(b * VC + vc) % 3].dma_start(
                    out=out[b, vc * 128:(vc + 1) * 128, :], in_=o_sb
                )

    nc.compile()
    return nc


def kernel(v, q, Wv, bv, Wq, bq, h_mat, h_bias):
    import ml_dtypes
    from concourse import bass_utils

    bf16 = ml_dtypes.bfloat16

    if "nc" not in _CACHE:
        _CACHE["nc"] = _build_nc()
    nc = _CACHE["nc"]

    v = np.asarray(v, dtype=np.float32)
    q = np.asarray(q, dtype=np.float32)
    Wv = np.asarray(Wv, dtype=np.float32)
    Wq = np.asarray(Wq, dtype=np.float32)
    bv = np.asarray(bv, dtype=np.float32)
    bq = np.asarray(bq, dtype=np.float32)
    h_mat = np.asarray(h_mat, dtype=np.float32)
    h_bias = np.asarray(h_bias, dtype=np.float32)

    vT = np.ascontiguousarray(v.transpose(0, 2, 1)).astype(bf16)      # (B, 2048, 512)
    WvT = np.ascontiguousarray(Wv.T).astype(bf16)                     # (2048, 1536)
    WqT = np.ascontiguousarray(Wq.T).astype(bf16)                     # (1024, 1536)
    bvT = np.ascontiguousarray(bv.reshape(JC, 128).T)                 # (128, 12)
    bqT = np.ascontiguousarray(bq.reshape(JC, 128).T)
    # hm[p, jc, h] = h_mat[h, jc*128+p]
    hmP = np.ascontiguousarray(
        h_mat.reshape(H_OUT, JC, 128).transpose(2, 1, 0)).astype(bf16)
    hbB = np.broadcast_to(np.repeat(h_bias, NQ)[None, :], (128, H_OUT * NQ))
    biasP = np.ascontiguousarray(
        np.concatenate([bvT, bqT, hbB], axis=1)).astype(np.float32)

    in_maps = []
    for c in range(N_CORES):
        bs = slice(BPC * c, BPC * (c + 1))
        qTc = np.ascontiguousarray(
            q[bs].transpose(2, 0, 1).reshape(Q_DIM, BPC * NQ)
        ).astype(bf16)
        in_maps.append({
            "vT": vT[bs],
            "qT": qTc,
            "WvT": WvT,
            "WqT": WqT,
            "bias": biasP,
            "hm": hmP,
        })

    res = bass_utils.run_bass_kernel_spmd(nc, in_maps, list(range(N_CORES)))
    outs = np.concatenate(
        [np.asarray(res.results[c]["out"]).astype(np.float32) for c in range(N_CORES)],
        axis=0,
    )
    # (32, 512, 1024) -> (32, 512, 8, 128) -> (32, 8, 512, 128)
    logits = outs.reshape(B, NV, H_OUT, NQ).transpose(0, 2, 1, 3)
    return np.ascontiguousarray(logits)


# revision 16
# speedup vs baseline: 1.1864x; 1.1834x over previous
"""Trainium2 Bass kernel for nn_BCNet: three-way low-rank bilinear net.

reference:
  v_ = relu(v @ Wv.T + bv)            # (B, NV, HK)
  q_ = relu(q @ Wq.T + bq)            # (B, NQ, HK)
  logits = einsum('hk,bvk,bqk->bhvq', h_mat, v_, q_) + h_bias

Sharding: data-parallel over batch, 4 batch items per core (8 cores).
All matmuls in bf16 with fp32 PSUM accumulation.

The kernel is PE-streaming-bound (639,700 moving columns = 266.5us at
2.4 GHz); everything else is engineered to hide behind that stream:
 - few, large DMAs (issue cost ~0.7us each; teardown scales with count)
 - critical first chunks issued first, spread over 4 issue engines
 - PE warm-up matmuls on a memset scratch tile (HAM un-throttle early)
 - qh built with broadcast tensor_tensor (48 DVE ops, not 384)
 - stage C reuses each vact weight tile for both nh halves
 - output stored as bf16, one DMA per (b, vc); host upcasts

Host prep per core:
  vT   (4, 2048, 512) bf16  : v[b].T per batch item
  qT   (1024, 512)    bf16  : q[4c:4c+4] transposed+stacked, cols = b*128+q
  WvT  (2048, 1536)   bf16
  WqT  (1024, 1536)   bf16
  bias (128, 12+12+1024) f32 : [bvT | bqT | hb broadcast]
  hm   (128, 12, 8) bf16 : h_mat[h, jc*128+p]
Device output per core: out (4, 512, 1024) bf16, cols = h*128+q.
Host post: concat -> (32, 512, 8, 128) -> transpose -> (32, 8, 512, 128).
"""

import numpy as np

B, NV, NQ = 32, 512, 128
V_DIM, Q_DIM, HK, H_OUT = 2048, 1024, 1536, 8
N_CORES = 8
BPC = B // N_CORES          # 4 batch items per core
JC = HK // 128              # 12 k-chunks
DCV = V_DIM // 128          # 16 contraction chunks for v
DCQ = Q_DIM // 128          # 8 contraction chunks for q
VC = NV // 128              # 4 v-chunks

_CACHE = {}


def _build_nc():
    import concourse.tile as tile
    from concourse import bacc, mybir
    from contextlib import ExitStack

    bf16 = mybir.dt.bfloat16
    f32 = mybir.dt.float32

    nc = bacc.Bacc()

    vT = nc.declare_dram_parameter("vT", [BPC, V_DIM, NV], bf16, isOutput=False)
    qT = nc.declare_dram_parameter("qT", [Q_DIM, BPC * NQ], bf16, isOutput=False)
    WvT = nc.declare_dram_parameter("WvT", [V_DIM, HK], bf16, isOutput=False)
    WqT = nc.declare_dram_parameter("WqT", [Q_DIM, HK], bf16, isOutput=False)
    bias = nc.declare_dram_parameter("bias", [128, 2 * JC + H_OUT * NQ], f32, isOutput=False)
    hm = nc.declare_dram_parameter("hm", [128, JC, H_OUT], bf16, isOutput=False)
    out = nc.declare_dram_parameter("out", [BPC, NV, H_OUT * NQ], bf16, isOutput=True)

    with ExitStack() as ctx:
        tc = ctx.enter_context(tile.TileContext(nc))
        consts = ctx.enter_context(tc.tile_pool(name="consts", bufs=1))
        qpool = ctx.enter_context(tc.tile_pool(name="qpool", bufs=1))
        vin = ctx.enter_context(tc.tile_pool(name="vin", bufs=2))
        vact = ctx.enter_context(tc.tile_pool(name="vact", bufs=2))
        qhp = ctx.enter_context(tc.tile_pool(name="qhp", bufs=1))
        outp = ctx.enter_context(tc.tile_pool(name="outp", bufs=3))
        ps = ctx.enter_context(tc.tile_pool(name="ps", bufs=8, space="PSUM"))

        # ---- PE warm-up: a few matmuls on zeroed scratch so the HAM
        # un-throttles (K=8/8) before the first real matmul arrives.
        scratch = consts.tile([128, 8 + 512], bf16)
        nc.gpsimd.memset(scratch, 0.0)
        ps_warm = ps.tile([8, 512], f32, tag="ps", name="warm")
        for _ in range(6):
            nc.tensor.matmul(ps_warm, lhsT=scratch[:, 0:8], rhs=scratch[:, 8:520],
                             start=True, stop=True)

        # ---- input DMAs. Three issue rings (sync/scalar HWDGE, gpsimd
        # SWDGE) share ~358 GB/s of HBM bandwidth; SDMA engines
        # round-robin rings at packet granularity, so bulk transfers
        # issued early starve the critical stream. Order of need:
        # Wq+qT (stage B, ~10us) -> Wv block0 + vT[0] (stage A start,
        # ~30us) -> Wv blocks 1-2 -> vT[1..3]/h_bias. Later transfers
        # are gated behind compute milestones via tiny copies.
        qT_r = qT.rearrange("(d p) n -> p d n", p=128)
        qt_sb = qpool.tile([128, DCQ, BPC * NQ], bf16)
        WqT_r = WqT.rearrange("(d p) j -> p d j", p=128)
        wq_sb = consts.tile([128, DCQ, HK], bf16)
        # B's first-group inputs (2 MB needed within ~7us of B's start)
        # split across BOTH HWDGE rings; each ring then continues with
        # later-needed blocks in consumption order. Wv block 0 and vT[0]
        # ride the gpsimd ring from the start (needed when A begins).
        WvT_r = WvT.rearrange("(d p) j -> p d j", p=128)
        wv_sb = consts.tile([128, DCV, HK], bf16)
        # Sync ring: qT d0-3, Wq j0 d4-7, Wq j-block 1, Wv j-block 1.
        nc.sync.dma_start(out=qt_sb[:, 0:4, :], in_=qT_r[:, 0:4, :])
        nc.sync.dma_start(out=wq_sb[:, 4:DCQ, 0:512], in_=WqT_r[:, 4:DCQ, 0:512])
        nc.sync.dma_start(out=wq_sb[:, :, 512:1024], in_=WqT_r[:, :, 512:1024])
        nc.sync.dma_start(out=wv_sb[:, 0:8, 512:1024], in_=WvT_r[:, 0:8, 512:1024])
        nc.sync.dma_start(out=wv_sb[:, 8:DCV, 512:1024], in_=WvT_r[:, 8:DCV, 512:1024])
        # Scalar ring: Wq j0 d0-3, qT d4-7, Wq j-block 2, Wv j-block 2.
        nc.scalar.dma_start(out=wq_sb[:, 0:4, 0:512], in_=WqT_r[:, 0:4, 0:512])
        nc.scalar.dma_start(out=qt_sb[:, 4:DCQ, :], in_=qT_r[:, 4:DCQ, :])
        nc.scalar.dma_start(out=wq_sb[:, :, 1024:1536], in_=WqT_r[:, :, 1024:1536])
        nc.scalar.dma_start(out=wv_sb[:, 0:8, 1024:1536], in_=WvT_r[:, 0:8, 1024:1536])
        nc.scalar.dma_start(out=wv_sb[:, 8:DCV, 1024:1536], in_=WvT_r[:, 8:DCV, 1024:1536])
        # GpSimd ring: consts, vT[0], Wv j-block 0.
        bias_sb = consts.tile([128, 2 * JC + H_OUT * NQ], f32)
        hm_sb = consts.tile([128, JC, H_OUT], bf16)
        nc.gpsimd.dma_start(out=bias_sb[:, 0:2 * JC], in_=bias[:, 0:2 * JC])
        nc.gpsimd.dma_start(out=hm_sb, in_=hm[:, :, :])
        bv_sb = bias_sb[:, 0:JC]
        bq_sb = bias_sb[:, JC:2 * JC]
        hb_sb = bias_sb[:, 2 * JC:]
        vt0_sb = vin.tile([128, DCV, NV], bf16, tag="vt", name="vt0")
        vT0_r = vT[0].rearrange("(d p) n -> p d n", p=128)
        nc.gpsimd.dma_start(out=vt0_sb[:, 0:8, :], in_=vT0_r[:, 0:8, :])
        nc.gpsimd.dma_start(out=vt0_sb[:, 8:DCV, :], in_=vT0_r[:, 8:DCV, :])
        nc.gpsimd.dma_start(out=wv_sb[:, 0:8, 0:512], in_=WvT_r[:, 0:8, 0:512])
        nc.gpsimd.dma_start(out=wv_sb[:, 8:DCV, 0:512], in_=WvT_r[:, 8:DCV, 0:512])

        # ---- stage B: q_ = relu(q @ Wq.T + bq), all 4 b at once ----
        # j-quads rotating over the shared 8-bank psum pool.
        qact_sb = qpool.tile([128, JC, BPC * NQ], bf16)
        for jg in range(0, JC, 4):
            pss = [ps.tile([128, BPC * NQ], f32, tag="ps", name=f"psB{jg}_{i}") for i in range(4)]
            for d in range(DCQ):
                for ji in range(4):
                    j = jg + ji
                    nc.tensor.matmul(
                        pss[ji],
                        lhsT=wq_sb[:, d, j * 128:(j + 1) * 128],
                        rhs=qt_sb[:, d, :],
                        start=(d == 0),
                        stop=(d == DCQ - 1),
                    )
            for ji in range(4):
                j = jg + ji
                nc.scalar.activation(
                    out=qact_sb[:, j, :],
                    in_=pss[ji],
                    func=mybir.ActivationFunctionType.Relu,
                    bias=bq_sb[:, j:j + 1],
                    scale=1.0,
                )

        gscr = consts.tile([128, 8], bf16)
        vt_tiles = {0: vt0_sb}
        for b in range(BPC):
            # ---- Qh[b][k, (h,q')] = q_[k, b*128+q'] * h_mat[h, k] ----
            # one broadcast tensor_tensor per j (DVE), not one per (j,h)
            qh_sb = qhp.tile([128, JC, H_OUT, NQ], bf16, tag="qh")
            for j in range(JC):
                nc.vector.tensor_mul(
                    qh_sb[:, j],
                    qact_sb[:, j, b * NQ:(b + 1) * NQ].unsqueeze(1).broadcast_to((128, H_OUT, NQ)),
                    hm_sb[:, j, :].unsqueeze(2).broadcast_to((128, H_OUT, NQ)),
                )

            # ---- stage A: v_[b] = relu(v[b] @ Wv.T + bv), transposed layout
            vt_sb = vt_tiles[b]
            vact_sb = vact.tile([128, JC, NV], bf16, tag="vact")
            for jg in range(0, JC, 4):
                pss = [ps.tile([128, NV], f32, tag="ps", name=f"psA{b}_{jg}_{i}") for i in range(4)]
                for d in range(DCV):
                    for ji in range(4):
                        j = jg + ji
                        nc.tensor.matmul(
                            pss[ji],
                            lhsT=wv_sb[:, d, j * 128:(j + 1) * 128],
                            rhs=vt_sb[:, d, :],
                            start=(d == 0),
                            stop=(d == DCV - 1),
                        )
                for ji in range(4):
                    j = jg + ji
                    nc.scalar.activation(
                        out=vact_sb[:, j, :],
                        in_=pss[ji],
                        func=mybir.ActivationFunctionType.Relu,
                        bias=bv_sb[:, j:j + 1],
                        scale=1.0,
                    )

            # prefetch next batch's vT on gpsimd, gated behind this
            # batch's first A-activation (ahead of C's gpsimd stores).
            if b + 1 < BPC:
                nvt = vin.tile([128, DCV, NV], bf16, tag="vt")
                vT_r = vT[b + 1].rearrange("(d p) n -> p d n", p=128)
                nc.gpsimd.tensor_copy(gscr[:, b + 1:b + 2], vact_sb[:, 0, 0:1])
                nc.gpsimd.dma_start(out=nvt, in_=vT_r)
                vt_tiles[b + 1] = nvt
            if b == 0:
                # h_bias after vT[1]: needed by C(b0)'s adds (~80us).
                nc.gpsimd.dma_start(out=bias_sb[:, 2 * JC:], in_=bias[:, 2 * JC:])

            # ---- stage C: logits[b] = v_[b] @ Qh[b] (contract over k)
            # j outer / nh inner: each vact weight tile loads once for
            # both nh halves; two 512-wide psum tiles per vc.
            engs = [nc.sync, nc.gpsimd, nc.scalar]
            for vc in range(VC):
                pos = [ps.tile([128, H_OUT * NQ // 2], f32, tag="ps",
                               name=f"psC{b}_{vc}_{nh}") for nh in range(2)]
                for j in range(JC):
                    for nh in range(2):
                        nc.tensor.matmul(
                            pos[nh],
                            lhsT=vact_sb[:, j, vc * 128:(vc + 1) * 128],
                            rhs=qh_sb[:, j, nh * 4:(nh + 1) * 4, :],
                            start=(j == 0),
                            stop=(j == JC - 1),
                        )
                o_sb = outp.tile([128, H_OUT * NQ], bf16, tag="osb")
                for nh in range(2):
                    sl = slice(nh * 512, (nh + 1) * 512)
                    nc.vector.tensor_add(o_sb[:, sl], pos[nh], hb_sb[:, sl])
                engs[(b * VC + vc) % 3].dma_start(
                    out=out[b, vc * 128:(vc + 1) * 128, :], in_=o_sb
                )

    nc.compile()
    return nc


def kernel(v, q, Wv, bv, Wq, bq, h_mat, h_bias):
    import ml_dtypes
    from concourse import bass_utils

    bf16 = ml_dtypes.bfloat16

    if "nc" not in _CACHE:
        _CACHE["nc"] = _build_nc()
    nc = _CACHE["nc"]

    v = np.asarray(v, dtype=np.float32)
    q = np.asarray(q, dtype=np.float32)
    Wv = np.asarray(Wv, dtype=np.float32)
    Wq = np.asarray(Wq, dtype=np.float32)
    bv = np.asarray(bv, dtype=np.float32)
    bq = np.asarray(bq, dtype=np.float32)
    h_mat = np.asarray(h_mat, dtype=np.float32)
    h_bias = np.asarray(h_bias, dtype=np.float32)

    vT = np.ascontiguousarray(v.transpose(0, 2, 1)).astype(bf16)      # (B, 2048, 512)
    WvT = np.ascontiguousarray(Wv.T).astype(bf16)                     # (2048, 1536)
    WqT = np.ascontiguousarray(Wq.T).astype(bf16)                     # (1024, 1536)
    bvT = np.ascontiguousarray(bv.reshape(JC, 128).T)                 # (128, 12)
    bqT = np.ascontiguousarray(bq.reshape(JC, 128).T)
    # hm[p, jc, h] = h_mat[h, jc*128+p]
    hmP = np.ascontiguousarray(
        h_mat.reshape(H_OUT, JC, 128).transpose(2, 1, 0)).astype(bf16)
    hbB = np.broadcast_to(np.repeat(h_bias, NQ)[None, :], (128, H_OUT * NQ))
    biasP = np.ascontiguousarray(
        np.concatenate([bvT, bqT, hbB], axis=1)).astype(np.float32)

    in_maps = []
    for c in range(N_CORES):
        bs = slice(BPC * c, BPC * (c + 1))
        qTc = np.ascontiguousarray(
            q[bs].transpose(2, 0, 1).reshape(Q_DIM, BPC * NQ)
        ).astype(bf16)
        in_maps.append({
            "vT": vT[bs],
            "qT": qTc,
            "WvT": WvT,
            "WqT": WqT,
            "bias": biasP,
            "hm": hmP,
        })

    res = bass_utils.run_bass_kernel_spmd(nc, in_maps, list(range(N_CORES)))
    outs = np.concatenate(
        [np.asarray(res.results[c]["out"]).astype(np.float32) for c in range(N_CORES)],
        axis=0,
    )
    # (32, 512, 1024) -> (32, 512, 8, 128) -> (32, 8, 512, 128)
    logits = outs.reshape(B, NV, H_OUT, NQ).transpose(0, 2, 1, 3)
    return np.ascontiguousarray(logits)


# revision 19
# speedup vs baseline: 1.1875x; 1.0010x over previous
"""Trainium2 Bass kernel for nn_BCNet: three-way low-rank bilinear net.

reference:
  v_ = relu(v @ Wv.T + bv)            # (B, NV, HK)
  q_ = relu(q @ Wq.T + bq)            # (B, NQ, HK)
  logits = einsum('hk,bvk,bqk->bhvq', h_mat, v_, q_) + h_bias

Sharding: data-parallel over batch, 4 batch items per core (8 cores).
All matmuls in bf16 with fp32 PSUM accumulation.

The kernel is PE-streaming-bound (639,700 moving columns = 266.5us at
2.4 GHz); everything else is engineered to hide behind that stream:
 - few, large DMAs (issue cost ~0.7us each; teardown scales with count)
 - critical first chunks issued first, spread over 4 issue engines
 - PE warm-up matmuls on a memset scratch tile (HAM un-throttle early)
 - qh built with broadcast tensor_tensor (48 DVE ops, not 384)
 - stage C reuses each vact weight tile for both nh halves
 - output stored as bf16, one DMA per (b, vc); host upcasts

Host prep per core:
  vT   (4, 2048, 512) bf16  : v[b].T per batch item
  qT   (1024, 512)    bf16  : q[4c:4c+4] transposed+stacked, cols = b*128+q
  WvT  (2048, 1536)   bf16
  WqT  (1024, 1536)   bf16
  bias (128, 12+12+1024) f32 : [bvT | bqT | hb broadcast]
  hm   (128, 12, 8) bf16 : h_mat[h, jc*128+p]
Device output per core: out (4, 512, 1024) bf16, cols = h*128+q.
Host post: concat -> (32, 512, 8, 128) -> transpose -> (32, 8, 512, 128).
"""

import numpy as np

B, NV, NQ = 32, 512, 128
V_DIM, Q_DIM, HK, H_OUT = 2048, 1024, 1536, 8
N_CORES = 8
BPC = B // N_CORES          # 4 batch items per core
JC = HK // 128              # 12 k-chunks
DCV = V_DIM // 128          # 16 contraction chunks for v
DCQ = Q_DIM // 128          # 8 contraction chunks for q
VC = NV // 128              # 4 v-chunks

_CACHE = {}


def _build_nc():
    import concourse.tile as tile
    from concourse import bacc, mybir
    from contextlib import ExitStack

    bf16 = mybir.dt.bfloat16
    f32 = mybir.dt.float32

    nc = bacc.Bacc()

    vT = nc.declare_dram_parameter("vT", [BPC, V_DIM, NV], bf16, isOutput=False)
    qT = nc.declare_dram_parameter("qT", [Q_DIM, BPC * NQ], bf16, isOutput=False)
    WvT = nc.declare_dram_parameter("WvT", [V_DIM, HK], bf16, isOutput=False)
    WqT = nc.declare_dram_parameter("WqT", [Q_DIM, HK], bf16, isOutput=False)
    bias = nc.declare_dram_parameter("bias", [128, 2 * JC + H_OUT * NQ], f32, isOutput=False)
    hm = nc.declare_dram_parameter("hm", [128, JC, H_OUT], bf16, isOutput=False)
    out = nc.declare_dram_parameter("out", [BPC, NV, H_OUT * NQ], bf16, isOutput=True)

    with ExitStack() as ctx:
        tc = ctx.enter_context(tile.TileContext(nc))
        consts = ctx.enter_context(tc.tile_pool(name="consts", bufs=1))
        qpool = ctx.enter_context(tc.tile_pool(name="qpool", bufs=1))
        vin = ctx.enter_context(tc.tile_pool(name="vin", bufs=2))
        vact = ctx.enter_context(tc.tile_pool(name="vact", bufs=2))
        qhp = ctx.enter_context(tc.tile_pool(name="qhp", bufs=1))
        outp = ctx.enter_context(tc.tile_pool(name="outp", bufs=3))
        ps = ctx.enter_context(tc.tile_pool(name="ps", bufs=8, space="PSUM"))

        # ---- PE warm-up: a few matmuls on zeroed scratch so the HAM
        # un-throttles (K=8/8) before the first real matmul arrives.
        scratch = consts.tile([128, 8 + 512], bf16)
        nc.gpsimd.memset(scratch, 0.0)
        ps_warm = ps.tile([8, 512], f32, tag="ps", name="warm")
        for _ in range(8):
            nc.tensor.matmul(ps_warm, lhsT=scratch[:, 0:8], rhs=scratch[:, 8:520],
                             start=True, stop=True)

        # ---- input DMAs. Three issue rings (sync/scalar HWDGE, gpsimd
        # SWDGE) share ~358 GB/s of HBM bandwidth; SDMA engines
        # round-robin rings at packet granularity, so bulk transfers
        # issued early starve the critical stream. Order of need:
        # Wq+qT (stage B, ~10us) -> Wv block0 + vT[0] (stage A start,
        # ~30us) -> Wv blocks 1-2 -> vT[1..3]/h_bias. Later transfers
        # are gated behind compute milestones via tiny copies.
        qT_r = qT.rearrange("(d p) n -> p d n", p=128)
        qt_sb = qpool.tile([128, DCQ, BPC * NQ], bf16)
        WqT_r = WqT.rearrange("(d p) j -> p d j", p=128)
        wq_sb = consts.tile([128, DCQ, HK], bf16)
        # B's first-group inputs (2 MB needed within ~7us of B's start)
        # split across BOTH HWDGE rings; each ring then continues with
        # later-needed blocks in consumption order. Wv block 0 and vT[0]
        # ride the gpsimd ring from the start (needed when A begins).
        WvT_r = WvT.rearrange("(d p) j -> p d j", p=128)
        wv_sb = consts.tile([128, DCV, HK], bf16)
        vt0_sb = vin.tile([128, DCV, NV], bf16, tag="vt", name="vt0")
        vT0_r = vT[0].rearrange("(d p) n -> p d n", p=128)
        # Sync ring: qT d0-3, Wq j0 d4-7, Wq j-block 1, vT0 d0-7, Wv j-block 1.
        nc.sync.dma_start(out=qt_sb[:, 0:4, :], in_=qT_r[:, 0:4, :])
        nc.sync.dma_start(out=wq_sb[:, 4:DCQ, 0:512], in_=WqT_r[:, 4:DCQ, 0:512])
        nc.sync.dma_start(out=wq_sb[:, :, 512:1024], in_=WqT_r[:, :, 512:1024])
        nc.sync.dma_start(out=vt0_sb[:, 0:8, :], in_=vT0_r[:, 0:8, :])
        nc.sync.dma_start(out=wv_sb[:, 0:8, 512:1024], in_=WvT_r[:, 0:8, 512:1024])
        nc.sync.dma_start(out=wv_sb[:, 8:DCV, 512:1024], in_=WvT_r[:, 8:DCV, 512:1024])
        # Scalar ring: Wq j0 d0-3, qT d4-7, Wq j-block 2, vT0 d8-15, Wv j-block 2.
        nc.scalar.dma_start(out=wq_sb[:, 0:4, 0:512], in_=WqT_r[:, 0:4, 0:512])
        nc.scalar.dma_start(out=qt_sb[:, 4:DCQ, :], in_=qT_r[:, 4:DCQ, :])
        nc.scalar.dma_start(out=wq_sb[:, :, 1024:1536], in_=WqT_r[:, :, 1024:1536])
        nc.scalar.dma_start(out=vt0_sb[:, 8:DCV, :], in_=vT0_r[:, 8:DCV, :])
        nc.scalar.dma_start(out=wv_sb[:, 0:8, 1024:1536], in_=WvT_r[:, 0:8, 1024:1536])
        nc.scalar.dma_start(out=wv_sb[:, 8:DCV, 1024:1536], in_=WvT_r[:, 8:DCV, 1024:1536])
        # GpSimd ring: small consts now; Wv j-block 0 gated behind B's
        # first activation (emitted after stage B below).
        bias_sb = consts.tile([128, 2 * JC + H_OUT * NQ], f32)
        hm_sb = consts.tile([128, JC, H_OUT], bf16)
        nc.gpsimd.dma_start(out=bias_sb[:, 0:2 * JC], in_=bias[:, 0:2 * JC])
        nc.gpsimd.dma_start(out=hm_sb, in_=hm[:, :, :])
        bv_sb = bias_sb[:, 0:JC]
        bq_sb = bias_sb[:, JC:2 * JC]
        hb_sb = bias_sb[:, 2 * JC:]

        # ---- stage B: q_ = relu(q @ Wq.T + bq), all 4 b at once ----
        # j-quads rotating over the shared 8-bank psum pool.
        qact_sb = qpool.tile([128, JC, BPC * NQ], bf16)
        for jg in range(0, JC, 4):
            pss = [ps.tile([128, BPC * NQ], f32, tag="ps", name=f"psB{jg}_{i}") for i in range(4)]
            for d in range(DCQ):
                for ji in range(4):
                    j = jg + ji
                    nc.tensor.matmul(
                        pss[ji],
                        lhsT=wq_sb[:, d, j * 128:(j + 1) * 128],
                        rhs=qt_sb[:, d, :],
                        start=(d == 0),
                        stop=(d == DCQ - 1),
                    )
            for ji in range(4):
                j = jg + ji
                nc.scalar.activation(
                    out=qact_sb[:, j, :],
                    in_=pss[ji],
                    func=mybir.ActivationFunctionType.Relu,
                    bias=bq_sb[:, j:j + 1],
                    scale=1.0,
                )

        # Wv j-block 0 (first consumed by stage A) on the gpsimd ring,
        # gated behind B's first activation so B's inputs own the start.
        gscr = consts.tile([128, 8], bf16)
        nc.gpsimd.tensor_copy(gscr[:, 0:1], qact_sb[:, 0, 0:1])
        nc.gpsimd.dma_start(out=wv_sb[:, 0:8, 0:512], in_=WvT_r[:, 0:8, 0:512])
        nc.gpsimd.dma_start(out=wv_sb[:, 8:DCV, 0:512], in_=WvT_r[:, 8:DCV, 0:512])

        vt_tiles = {0: vt0_sb}
        for b in range(BPC):
            # ---- Qh[b][k, (h,q')] = q_[k, b*128+q'] * h_mat[h, k] ----
            # one broadcast tensor_tensor per j (DVE), not one per (j,h)
            qh_sb = qhp.tile([128, JC, H_OUT, NQ], bf16, tag="qh")
            for j in range(JC):
                nc.vector.tensor_mul(
                    qh_sb[:, j],
                    qact_sb[:, j, b * NQ:(b + 1) * NQ].unsqueeze(1).broadcast_to((128, H_OUT, NQ)),
                    hm_sb[:, j, :].unsqueeze(2).broadcast_to((128, H_OUT, NQ)),
                )

            # ---- stage A: v_[b] = relu(v[b] @ Wv.T + bv), transposed layout
            vt_sb = vt_tiles[b]
            vact_sb = vact.tile([128, JC, NV], bf16, tag="vact")
            for jg in range(0, JC, 4):
                pss = [ps.tile([128, NV], f32, tag="ps", name=f"psA{b}_{jg}_{i}") for i in range(4)]
                for d in range(DCV):
                    for ji in range(4):
                        j = jg + ji
                        nc.tensor.matmul(
                            pss[ji],
                            lhsT=wv_sb[:, d, j * 128:(j + 1) * 128],
                            rhs=vt_sb[:, d, :],
                            start=(d == 0),
                            stop=(d == DCV - 1),
                        )
                for ji in range(4):
                    j = jg + ji
                    nc.scalar.activation(
                        out=vact_sb[:, j, :],
                        in_=pss[ji],
                        func=mybir.ActivationFunctionType.Relu,
                        bias=bv_sb[:, j:j + 1],
                        scale=1.0,
                    )

            # prefetch next batch's vT on gpsimd, gated behind this
            # batch's first A-activation (ahead of C's gpsimd stores).
            if b + 1 < BPC:
                nvt = vin.tile([128, DCV, NV], bf16, tag="vt")
                vT_r = vT[b + 1].rearrange("(d p) n -> p d n", p=128)
                nc.gpsimd.tensor_copy(gscr[:, b + 1:b + 2], vact_sb[:, 0, 0:1])
                nc.gpsimd.dma_start(out=nvt, in_=vT_r)
                vt_tiles[b + 1] = nvt
            if b == 0:
                # h_bias after vT[1]: needed by C(b0)'s adds (~80us).
                nc.gpsimd.dma_start(out=bias_sb[:, 2 * JC:], in_=bias[:, 2 * JC:])

            # ---- stage C: logits[b] = v_[b] @ Qh[b] (contract over k)
            # j outer / nh inner: each vact weight tile loads once for
            # both nh halves; two 512-wide psum tiles per vc.
            engs = [nc.sync, nc.gpsimd, nc.scalar]
            for vc in range(VC):
                pos = [ps.tile([128, H_OUT * NQ // 2], f32, tag="ps",
                               name=f"psC{b}_{vc}_{nh}") for nh in range(2)]
                for j in range(JC):
                    for nh in range(2):
                        nc.tensor.matmul(
                            pos[nh],
                            lhsT=vact_sb[:, j, vc * 128:(vc + 1) * 128],
                            rhs=qh_sb[:, j, nh * 4:(nh + 1) * 4, :],
                            start=(j == 0),
                            stop=(j == JC - 1),
                        )
                o_sb = outp.tile([128, H_OUT * NQ], bf16, tag="osb")
                for nh in range(2):
                    sl = slice(nh * 512, (nh + 1) * 512)
                    nc.vector.tensor_add(o_sb[:, sl], pos[nh], hb_sb[:, sl])
                engs[(b * VC + vc) % 3].dma_start(
                    out=out[b, vc * 128:(vc + 1) * 128, :], in_=o_sb
                )

    nc.compile()
    return nc


def kernel(v, q, Wv, bv, Wq, bq, h_mat, h_bias):
    import ml_dtypes
    from concourse import bass_utils

    bf16 = ml_dtypes.bfloat16

    if "nc" not in _CACHE:
        _CACHE["nc"] = _build_nc()
    nc = _CACHE["nc"]

    v = np.asarray(v, dtype=np.float32)
    q = np.asarray(q, dtype=np.float32)
    Wv = np.asarray(Wv, dtype=np.float32)
    Wq = np.asarray(Wq, dtype=np.float32)
    bv = np.asarray(bv, dtype=np.float32)
    bq = np.asarray(bq, dtype=np.float32)
    h_mat = np.asarray(h_mat, dtype=np.float32)
    h_bias = np.asarray(h_bias, dtype=np.float32)

    vT = np.ascontiguousarray(v.transpose(0, 2, 1)).astype(bf16)      # (B, 2048, 512)
    WvT = np.ascontiguousarray(Wv.T).astype(bf16)                     # (2048, 1536)
    WqT = np.ascontiguousarray(Wq.T).astype(bf16)                     # (1024, 1536)
    bvT = np.ascontiguousarray(bv.reshape(JC, 128).T)                 # (128, 12)
    bqT = np.ascontiguousarray(bq.reshape(JC, 128).T)
    # hm[p, jc, h] = h_mat[h, jc*128+p]
    hmP = np.ascontiguousarray(
        h_mat.reshape(H_OUT, JC, 128).transpose(2, 1, 0)).astype(bf16)
    hbB = np.broadcast_to(np.repeat(h_bias, NQ)[None, :], (128, H_OUT * NQ))
    biasP = np.ascontiguousarray(
        np.concatenate([bvT, bqT, hbB], axis=1)).astype(np.float32)

    in_maps = []
    for c in range(N_CORES):
        bs = slice(BPC * c, BPC * (c + 1))
        qTc = np.ascontiguousarray(
            q[bs].transpose(2, 0, 1).reshape(Q_DIM, BPC * NQ)
        ).astype(bf16)
        in_maps.append({
            "vT": vT[bs],
            "qT": qTc,
            "WvT": WvT,
            "WqT": WqT,
            "bias": biasP,
            "hm": hmP,
        })

    res = bass_utils.run_bass_kernel_spmd(nc, in_maps, list(range(N_CORES)))
    outs = np.concatenate(
        [np.asarray(res.results[c]["out"]).astype(np.float32) for c in range(N_CORES)],
        axis=0,
    )
    # (32, 512, 1024) -> (32, 512, 8, 128) -> (32, 8, 512, 128)
    logits = outs.reshape(B, NV, H_OUT, NQ).transpose(0, 2, 1, 3)
    return np.ascontiguousarray(logits)
